# revision 10
# baseline (speedup 1.0000x reference)
"""TRN2 Bass kernel for nn_AttentionSimilarityLinear (B=4, S=8192, D=1024, C=256).

Sharding: 8 cores = (batch b = core//2, seq half h = core%2), 4096 rows each.
Two SPMD launches:
  L1 (rejected pass): K = elu(R @ W_K)+1 masked; outputs per-core partial
     KV = K^T K (fp32r matmuls, fp32 PSUM) and K_sum partition-partials.
  host: reduce KV/K_sum over core pairs in fp64; split into fp32r hi/lo pairs.
  L2 (chosen pass): Q^T = elu(C @ W_Q)+1; T = (KVh+KVl) @ Q^T;
     num[s] = sum_d (T .* Q^T); den[s] = Q^T . (Ksh+Ksl); outputs num/den rows.
  host tail in fp64: row = max(num,eps)/max((den+eps)^2,eps); masked mean; tau; clip.

All matmuls use fp32r (operands RNE-rounded to 11 mantissa bits, full-speed PE).
The hidden states and weights are pre-rounded on host (exactly matching the HW
rounding, verified bit-exact) and shipped as fp32r DRAM tensors, so no on-chip
rounding passes are needed. The ill-conditioned (n*m-1) cancellation runs on
host in fp64; the hi/lo split of KV/K_sum keeps the row-stat matmuls at ~2^-22
effective precision, holding end-to-end tau error at ~0.02% (simulated).
"""
import os
import numpy as np

import concourse.bacc as bacc
import concourse.tile as tile
from concourse import mybir
from concourse.bass_utils import run_bass_kernel_spmd

B, S, D, C = 4, 8192, 1024, 256
SH = S // 2              # rows per core
N_CORES = 8
EPS = 1e-8

F32 = mybir.dt.float32
F32R = mybir.dt.float32r
Relu = mybir.ActivationFunctionType.Relu
Exp = mybir.ActivationFunctionType.Exp

# group geometry
GW = 512                 # rows per DMA group
NG = SH // GW            # 8 groups
NT = GW // 128           # 4 tiles of 128 rows per group
DC = D // 128            # 8 contraction chunks

TRACE = bool(os.environ.get("KERNEL_TRACE"))
LAST_EXEC_NS = {}
_NC_CACHE = {}


def _rne11(x):
    """Round fp32 array to 11 explicit mantissa bits, RNE (matches HW fp32r)."""
    x = np.ascontiguousarray(x, np.float32)
    u = x.view(np.uint32)
    shift = np.uint32(12)  # 23 - 11
    low = u & np.uint32((1 << 12) - 1)
    half = np.uint32(1 << 11)
    base = u >> shift
    rnd = base + (((low > half) | ((low == half) & ((base & 1) == 1)))).astype(np.uint32)
    return (rnd << shift).view(np.float32).copy()


def _split_hi_lo(x64):
    """fp64 array -> (hi, lo) fp32 arrays, both 11-bit-mantissa representable."""
    hi = _rne11(x64.astype(np.float32))
    lo = _rne11((x64 - hi.astype(np.float64)).astype(np.float32))
    return hi, lo


def _build_l1(apply_mask, reps=1):
    nc = bacc.Bacc()
    rt_d = nc.declare_dram_parameter("rt", [D, SH], F32R, isOutput=False)
    wk_d = nc.declare_dram_parameter("wk", [D, C], F32R, isOutput=False)
    if apply_mask:
        rmask_d = nc.declare_dram_parameter("rmask", [128, SH // 128], F32, isOutput=False)
    kv_d = nc.declare_dram_parameter("kv_part", [C, C], F32, isOutput=True)
    ksum_d = nc.declare_dram_parameter("ksum_acc", [128, C], F32, isOutput=True)

    with tile.TileContext(nc) as tc:
        with (
            tc.tile_pool(name="const", bufs=1) as const,
            tc.tile_pool(name="xin", bufs=3) as xin,
            tc.tile_pool(name="kt", bufs=3) as kt,
            tc.tile_pool(name="ps_k", bufs=4, space="PSUM") as ps_k_pool,
            tc.tile_pool(name="ps_kv", bufs=1, space="PSUM") as ps_kv_pool,
        ):
            wkr = const.tile([128, DC, C], F32R)
            nc.sync.dma_start(out=wkr, in_=wk_d[:].rearrange("(a p) c -> p a c", p=128))
            if apply_mask:
                rmask_sb = const.tile([128, SH // 128], F32)
                nc.sync.dma_start(out=rmask_sb, in_=rmask_d[:])

            ksum_acc = const.tile([128, C], F32)
            nc.vector.memset(ksum_acc, 0.0)

            kv_ps = [
                ps_kv_pool.tile([128, C], F32, tag=f"kv{cc}", name=f"kv_ps{cc}")
                for cc in range(2)
            ]

            for rep in range(reps):
              for g in range(NG):
                xtr = xin.tile([128, DC, GW], F32R, tag="xtr")
                nc.sync.dma_start(
                    out=xtr,
                    in_=rt_d[:, g * GW:(g + 1) * GW].rearrange("(a p) s -> p a s", p=128),
                )

                for t in range(NT):
                    ps_k = ps_k_pool.tile([128, C], F32)
                    for dc in range(DC):
                        nc.tensor.matmul(
                            ps_k,
                            lhsT=xtr[:, dc, t * 128:(t + 1) * 128],
                            rhs=wkr[:, dc, :],
                            start=(dc == 0),
                            stop=(dc == DC - 1),
                        )
                    idx = g * NT + t
                    gidx = rep * (SH // 128) + idx
                    # elu(x)+1 = relu(x) + exp(min(x, 0))
                    relu_t = kt.tile([128, C], F32, tag="relu")
                    nc.scalar.activation(out=relu_t, in_=ps_k, func=Relu)
                    mn_t = kt.tile([128, C], F32, tag="mn")
                    nc.vector.tensor_scalar_min(out=mn_t, in0=ps_k, scalar1=0.0)
                    ex_t = kt.tile([128, C], F32, tag="ex")
                    nc.scalar.activation(out=ex_t, in_=mn_t, func=Exp)
                    if apply_mask:
                        ke_t = kt.tile([128, C], F32, tag="ke")
                        nc.vector.tensor_add(out=ke_t, in0=relu_t, in1=ex_t)
                        kr = kt.tile([128, C], F32R, tag="kr")
                        nc.vector.tensor_scalar_mul(
                            out=kr, in0=ke_t, scalar1=rmask_sb[:, idx:idx + 1]
                        )
                    else:
                        kr = kt.tile([128, C], F32R, tag="kr")
                        nc.vector.tensor_add(out=kr, in0=relu_t, in1=ex_t)
                    nc.vector.tensor_add(out=ksum_acc, in0=ksum_acc, in1=kr)
                    first = gidx == 0
                    last = gidx == reps * (SH // 128) - 1
                    for cc in range(2):
                        nc.tensor.matmul(
                            kv_ps[cc],
                            lhsT=kr[:, cc * 128:(cc + 1) * 128],
                            rhs=kr[:, :],
                            start=first,
                            stop=last,
                        )

            for cc in range(2):
                kv_sb = kt.tile([128, C], F32, tag=f"kvout{cc}")
                nc.vector.tensor_copy(out=kv_sb, in_=kv_ps[cc])
                nc.sync.dma_start(out=kv_d[cc * 128:(cc + 1) * 128, :], in_=kv_sb)
            nc.sync.dma_start(out=ksum_d[:], in_=ksum_acc)

    nc.finalize()
    return nc


def _build_l2(reps=1):
    nc = bacc.Bacc()
    ct_d = nc.declare_dram_parameter("ct", [D, SH], F32R, isOutput=False)
    wq_d = nc.declare_dram_parameter("wq", [D, C], F32R, isOutput=False)
    kvh_d = nc.declare_dram_parameter("kvh", [C, C], F32R, isOutput=False)
    kvl_d = nc.declare_dram_parameter("kvl", [C, C], F32R, isOutput=False)
    ksh_d = nc.declare_dram_parameter("ksh", [128, 2], F32R, isOutput=False)
    ksl_d = nc.declare_dram_parameter("ksl", [128, 2], F32R, isOutput=False)
    num_d = nc.declare_dram_parameter("num_out", [1, SH], F32, isOutput=True)
    den_d = nc.declare_dram_parameter("den_out", [1, SH], F32, isOutput=True)

    with tile.TileContext(nc) as tc:
        with (
            tc.tile_pool(name="const", bufs=1) as const,
            tc.tile_pool(name="xin", bufs=3) as xin,
            tc.tile_pool(name="qt", bufs=2) as qt,
            tc.tile_pool(name="out", bufs=1) as outp,
            tc.tile_pool(name="ps_q", bufs=2, space="PSUM") as ps_q_pool,
            tc.tile_pool(name="ps_t", bufs=2, space="PSUM") as ps_t_pool,
            tc.tile_pool(name="ps_nd", bufs=2, space="PSUM") as ps_nd_pool,
        ):
            wqr = const.tile([128, DC, C], F32R)
            nc.sync.dma_start(out=wqr, in_=wq_d[:].rearrange("(a p) c -> p a c", p=128))

            # KV hi/lo: layout [p, cc, d] with c = cc*128 + p
            kv_r = []
            for name, d_in in (("kvh", kvh_d), ("kvl", kvl_d)):
                r = const.tile([128, 2, C], F32R, tag=f"{name}r", name=f"{name}_sb")
                nc.sync.dma_start(out=r, in_=d_in[:].rearrange("(a p) c -> p a c", p=128))
                kv_r.append(r)

            ks_r = []
            for name, d_in in (("ksh", ksh_d), ("ksl", ksl_d)):
                r = const.tile([128, 2], F32R, tag=f"{name}r", name=f"{name}_sb")
                nc.sync.dma_start(out=r, in_=d_in[:])
                ks_r.append(r)

            ones_sb = const.tile([128, 1], F32)
            nc.vector.memset(ones_sb, 1.0)
            ones_r = const.tile([128, 1], F32R)
            nc.vector.tensor_copy(out=ones_r, in_=ones_sb)

            num_sb = outp.tile([1, SH], F32)
            den_sb = outp.tile([1, SH], F32)

            for rep in range(reps):
              for g in range(NG):
                xtr = xin.tile([128, DC, GW], F32R, tag="xtr")
                nc.sync.dma_start(
                    out=xtr,
                    in_=ct_d[:, g * GW:(g + 1) * GW].rearrange("(a p) s -> p a s", p=128),
                )

                qtr = qt.tile([128, 2, GW], F32R, tag="qtr")
                for cc in range(2):
                    ps_q = ps_q_pool.tile([128, GW], F32)
                    for dc in range(DC):
                        nc.tensor.matmul(
                            ps_q,
                            lhsT=wqr[:, dc, cc * 128:(cc + 1) * 128],
                            rhs=xtr[:, dc, :],
                            start=(dc == 0),
                            stop=(dc == DC - 1),
                        )
                    relu_t = qt.tile([128, GW], F32, tag="relu")
                    nc.scalar.activation(out=relu_t, in_=ps_q, func=Relu)
                    mn_t = qt.tile([128, GW], F32, tag="mn")
                    nc.vector.tensor_scalar_min(out=mn_t, in0=ps_q, scalar1=0.0)
                    ex_t = qt.tile([128, GW], F32, tag="ex")
                    nc.scalar.activation(out=ex_t, in_=mn_t, func=Exp)
                    nc.vector.tensor_add(out=qtr[:, cc, :], in0=relu_t, in1=ex_t)

                ps_num = ps_nd_pool.tile([1, GW], F32, tag="num")
                ps_den = ps_nd_pool.tile([1, GW], F32, tag="den")
                for dd in range(2):
                    ps_t = ps_t_pool.tile([128, GW], F32)
                    mm = 0
                    for kv_t in kv_r:
                        for cc in range(2):
                            nc.tensor.matmul(
                                ps_t,
                                lhsT=kv_t[:, cc, dd * 128:(dd + 1) * 128],
                                rhs=qtr[:, cc, :],
                                start=(mm == 0),
                                stop=(mm == 3),
                            )
                            mm += 1
                    p2 = qt.tile([128, GW], F32R, tag="p2")
                    nc.vector.tensor_mul(out=p2, in0=ps_t, in1=qtr[:, dd, :])
                    nc.tensor.matmul(
                        ps_num, lhsT=ones_r, rhs=p2,
                        start=(dd == 0), stop=(dd == 1),
                    )
                mm = 0
                for ks_t in ks_r:
                    for cc in range(2):
                        nc.tensor.matmul(
                            ps_den,
                            lhsT=ks_t[:, cc:cc + 1],
                            rhs=qtr[:, cc, :],
                            start=(mm == 0),
                            stop=(mm == 3),
                        )
                        mm += 1
                nc.vector.tensor_copy(out=num_sb[:, g * GW:(g + 1) * GW], in_=ps_num)
                nc.vector.tensor_copy(out=den_sb[:, g * GW:(g + 1) * GW], in_=ps_den)

            nc.sync.dma_start(out=num_d[:], in_=num_sb)
            nc.sync.dma_start(out=den_d[:], in_=den_sb)

    nc.finalize()
    return nc


def _build_fused(apply_mask, reps=1):
    """Single launch: rejected pass -> pairwise AllReduce of (KV, Ksum) ->
    on-device hi/lo split -> chosen pass. Chosen-side DMA/projections overlap
    the rejected pass and the collective."""
    nc = bacc.Bacc()
    rt_d = nc.declare_dram_parameter("rt", [D, SH], F32R, isOutput=False)
    ct_d = nc.declare_dram_parameter("ct", [D, SH], F32R, isOutput=False)
    wk_d = nc.declare_dram_parameter("wk", [D, C], F32R, isOutput=False)
    wq_d = nc.declare_dram_parameter("wq", [D, C], F32R, isOutput=False)
    if apply_mask:
        rmask_d = nc.declare_dram_parameter("rmask", [128, SH // 128], F32, isOutput=False)
    num_d = nc.declare_dram_parameter("num_out", [1, SH], F32, isOutput=True)
    den_d = nc.declare_dram_parameter("den_out", [1, SH], F32, isOutput=True)
    cc_in = nc.dram_tensor("cc_in", [C + 1, C], F32)
    cc_out = nc.dram_tensor("cc_out", [C + 1, C], F32)
    groups = [[2 * b, 2 * b + 1] for b in range(B)]

    with tile.TileContext(nc) as tc:
        with (
            tc.tile_pool(name="const", bufs=1) as const,
            tc.tile_pool(name="xin", bufs=3) as xin,
            tc.tile_pool(name="kt", bufs=3) as kt,
            tc.tile_pool(name="qt", bufs=2) as qt,
            tc.tile_pool(name="out", bufs=1) as outp,
            tc.tile_pool(name="ps_mm", bufs=2, space="PSUM") as ps_mm_pool,
            tc.tile_pool(name="ps_kv", bufs=1, space="PSUM") as ps_kv_pool,
            tc.tile_pool(name="ps_t", bufs=2, space="PSUM") as ps_t_pool,
            tc.tile_pool(name="ps_nd", bufs=1, space="PSUM") as ps_nd_pool,
        ):
            wkr = const.tile([128, DC, C], F32R)
            nc.sync.dma_start(out=wkr, in_=wk_d[:].rearrange("(a p) c -> p a c", p=128))
            wqr = const.tile([128, DC, C], F32R)
            nc.sync.dma_start(out=wqr, in_=wq_d[:].rearrange("(a p) c -> p a c", p=128))
            if apply_mask:
                rmask_sb = const.tile([128, SH // 128], F32)
                nc.sync.dma_start(out=rmask_sb, in_=rmask_d[:])

            ones_sb = const.tile([128, 1], F32)
            nc.vector.memset(ones_sb, 1.0)
            ones_r = const.tile([128, 1], F32R)
            nc.vector.tensor_copy(out=ones_r, in_=ones_sb)

            ksum_acc = const.tile([128, C], F32)
            nc.vector.memset(ksum_acc, 0.0)
            kv_ps = [
                ps_kv_pool.tile([128, C], F32, tag=f"kv{cc}", name=f"kv_ps{cc}")
                for cc in range(2)
            ]

            # ---- rejected pass ----
            for rep in range(reps):
              for g in range(NG):
                xtr = xin.tile([128, DC, GW], F32R, tag="xtr")
                nc.sync.dma_start(
                    out=xtr,
                    in_=rt_d[:, g * GW:(g + 1) * GW].rearrange("(a p) s -> p a s", p=128),
                )
                for t in range(NT):
                    ps_k = ps_mm_pool.tile([128, GW], F32, tag="mm", name="ps_k")
                    ps_k = ps_k[:, :C]
                    for dc in range(DC):
                        nc.tensor.matmul(
                            ps_k,
                            lhsT=xtr[:, dc, t * 128:(t + 1) * 128],
                            rhs=wkr[:, dc, :],
                            start=(dc == 0),
                            stop=(dc == DC - 1),
                        )
                    idx = g * NT + t
                    gidx = rep * (SH // 128) + idx
                    relu_t = kt.tile([128, C], F32, tag="relu")
                    nc.scalar.activation(out=relu_t, in_=ps_k, func=Relu)
                    mn_t = kt.tile([128, C], F32, tag="mn")
                    nc.vector.tensor_scalar_min(out=mn_t, in0=ps_k, scalar1=0.0)
                    ex_t = kt.tile([128, C], F32, tag="ex")
                    nc.scalar.activation(out=ex_t, in_=mn_t, func=Exp)
                    if apply_mask:
                        ke_t = kt.tile([128, C], F32, tag="ke")
                        nc.vector.tensor_add(out=ke_t, in0=relu_t, in1=ex_t)
                        kr = kt.tile([128, C], F32R, tag="kr")
                        nc.vector.tensor_scalar_mul(
                            out=kr, in0=ke_t, scalar1=rmask_sb[:, idx:idx + 1]
                        )
                    else:
                        kr = kt.tile([128, C], F32R, tag="kr")
                        nc.gpsimd.tensor_add(out=kr, in0=relu_t, in1=ex_t)
                    nc.vector.tensor_add(out=ksum_acc, in0=ksum_acc, in1=kr)
                    first = gidx == 0
                    last = gidx == reps * (SH // 128) - 1
                    for cc in range(2):
                        nc.tensor.matmul(
                            kv_ps[cc],
                            lhsT=kr[:, cc * 128:(cc + 1) * 128],
                            rhs=kr[:, :],
                            start=first,
                            stop=last,
                        )

            # ---- reduce K stats across the core pair ----
            for cc in range(2):
                kv_sb = kt.tile([128, C], F32, tag=f"kvout{cc}")
                nc.vector.tensor_copy(out=kv_sb, in_=kv_ps[cc])
                nc.sync.dma_start(out=cc_in[cc * 128:(cc + 1) * 128, :], in_=kv_sb)
            # exact fp32 partition-reduce of ksum_acc
            ps_ks = ps_nd_pool.tile([1, C], F32, tag="num", name="ps_ks")
            nc.tensor.matmul(ps_ks, lhsT=ones_sb, rhs=ksum_acc, start=True, stop=True)
            ks_row = kt.tile([1, C], F32, tag="ksrow")
            nc.vector.tensor_copy(out=ks_row, in_=ps_ks)
            nc.sync.dma_start(out=cc_in[C:C + 1, :], in_=ks_row)

            nc.gpsimd.collective_compute(
                "AllReduce", mybir.AluOpType.add,
                replica_groups=groups,
                ins=[cc_in[:]], outs=[cc_out[:]],
            )

            kvsum_sb = const.tile([128, 2, C], F32)
            nc.sync.dma_start(
                out=kvsum_sb, in_=cc_out[0:C, :].rearrange("(a p) c -> p a c", p=128)
            )
            kvh_r = const.tile([128, 2, C], F32R)
            nc.vector.tensor_copy(out=kvh_r, in_=kvsum_sb)
            kvl_r = const.tile([128, 2, C], F32R)
            nc.vector.tensor_sub(out=kvl_r, in0=kvsum_sb, in1=kvh_r)
            kv_r = [kvh_r, kvl_r]

            ksc_sb = const.tile([128, 2], F32)
            nc.sync.dma_start(
                out=ksc_sb, in_=cc_out[C, :].rearrange("(a p) -> p a", p=128)
            )
            ksh_r = const.tile([128, 2], F32R)
            nc.vector.tensor_copy(out=ksh_r, in_=ksc_sb)
            ksl_r = const.tile([128, 2], F32R)
            nc.vector.tensor_sub(out=ksl_r, in0=ksc_sb, in1=ksh_r)
            ks_r = [ksh_r, ksl_r]

            # ---- chosen pass ----
            num_sb = outp.tile([1, SH], F32)
            den_sb = outp.tile([1, SH], F32)
            for rep in range(reps):
              for g in range(NG):
                xtc = xin.tile([128, DC, GW], F32R, tag="xtc", name="xtc")
                nc.sync.dma_start(
                    out=xtc,
                    in_=ct_d[:, g * GW:(g + 1) * GW].rearrange("(a p) s -> p a s", p=128),
                )
                qtr = qt.tile([128, 2, GW], F32R, tag="qtr")
                for cc in range(2):
                    ps_q = ps_mm_pool.tile([128, GW], F32, tag="mm", name="ps_q")
                    for dc in range(DC):
                        nc.tensor.matmul(
                            ps_q,
                            lhsT=wqr[:, dc, cc * 128:(cc + 1) * 128],
                            rhs=xtc[:, dc, :],
                            start=(dc == 0),
                            stop=(dc == DC - 1),
                        )
                    relu_t = qt.tile([128, GW], F32, tag="qrelu")
                    nc.scalar.activation(out=relu_t, in_=ps_q, func=Relu)
                    mn_t = qt.tile([128, GW], F32, tag="qmn")
                    nc.vector.tensor_scalar_min(out=mn_t, in0=ps_q, scalar1=0.0)
                    ex_t = qt.tile([128, GW], F32, tag="qex")
                    nc.scalar.activation(out=ex_t, in_=mn_t, func=Exp)
                    nc.gpsimd.tensor_add(out=qtr[:, cc, :], in0=relu_t, in1=ex_t)

                ps_num = ps_nd_pool.tile([1, GW], F32, tag="num", name="ps_num")
                ps_den = ps_nd_pool.tile([1, GW], F32, tag="den", name="ps_den")
                for dd in range(2):
                    ps_t = ps_t_pool.tile([128, GW], F32)
                    mm = 0
                    for kv_t in kv_r:
                        for cc in range(2):
                            nc.tensor.matmul(
                                ps_t,
                                lhsT=kv_t[:, cc, dd * 128:(dd + 1) * 128],
                                rhs=qtr[:, cc, :],
                                start=(mm == 0),
                                stop=(mm == 3),
                            )
                            mm += 1
                    p2 = qt.tile([128, GW], F32R, tag="p2")
                    nc.vector.tensor_mul(out=p2, in0=ps_t, in1=qtr[:, dd, :])
                    nc.tensor.matmul(
                        ps_num, lhsT=ones_r, rhs=p2,
                        start=(dd == 0), stop=(dd == 1),
                    )
                mm = 0
                for ks_t in ks_r:
                    for cc in range(2):
                        nc.tensor.matmul(
                            ps_den,
                            lhsT=ks_t[:, cc:cc + 1],
                            rhs=qtr[:, cc, :],
                            start=(mm == 0),
                            stop=(mm == 3),
                        )
                        mm += 1
                nc.vector.tensor_copy(out=num_sb[:, g * GW:(g + 1) * GW], in_=ps_num)
                nc.vector.tensor_copy(out=den_sb[:, g * GW:(g + 1) * GW], in_=ps_den)

            nc.sync.dma_start(out=num_d[:], in_=num_sb)
            nc.sync.dma_start(out=den_d[:], in_=den_sb)

    nc.finalize()
    return nc


def _run(nc, in_maps, label):
    kwargs = {}
    if TRACE:
        tmpdir = f"/tmp/kernel_trace_{label}"
        os.makedirs(tmpdir, exist_ok=True)
        kwargs = dict(trace=True, tmpdir=tmpdir)
    res = run_bass_kernel_spmd(nc, in_maps, core_ids=list(range(N_CORES)), **kwargs)
    if TRACE:
        LAST_EXEC_NS[label] = res.exec_time_ns
    return res.results


def kernel(chosen_hidden_states, rejected_hidden_states, chosen_mask,
           rejected_mask, W_Q, W_K):
    ch = np.ascontiguousarray(chosen_hidden_states, np.float32)
    rh = np.ascontiguousarray(rejected_hidden_states, np.float32)
    cm = np.ascontiguousarray(chosen_mask, np.float32)
    rm = np.ascontiguousarray(rejected_mask, np.float32)
    wqr = _rne11(np.ascontiguousarray(W_Q, np.float32))
    wkr = _rne11(np.ascontiguousarray(W_K, np.float32))

    def shard_r(x, core):
        b, h = divmod(core, 2)
        return _rne11(np.ascontiguousarray(x[b, h * SH:(h + 1) * SH, :].T))

    def mask_cols(m, core):
        b, h = divmod(core, 2)
        return np.ascontiguousarray(m[b, h * SH:(h + 1) * SH].reshape(SH // 128, 128).T)

    apply_mask = not np.all(rm == 1.0)

    # ---- preferred: fused single launch with pairwise AllReduce ----
    try:
        key = ("fused", apply_mask)
        nc = _NC_CACHE.get(key)
        if nc is None:
            nc = _build_fused(apply_mask)
            _NC_CACHE[key] = nc
        in_maps = []
        for c in range(N_CORES):
            m = {"rt": shard_r(rh, c), "ct": shard_r(ch, c),
                 "wk": wkr, "wq": wqr}
            if apply_mask:
                m["rmask"] = mask_cols(rm, c)
            in_maps.append(m)
        res = _run(nc, in_maps, "fused")
        return _host_tail(res, cm, rm)
    except Exception as e:  # pragma: no cover - fallback for safety
        import traceback
        traceback.print_exc()
        print(f"fused kernel failed ({e!r}); falling back to two-launch path")

    # ---- L1: rejected pass ----
    nc1 = _build_l1(apply_mask)
    in_maps1 = []
    for c in range(N_CORES):
        m = {"rt": shard_r(rh, c), "wk": wkr}
        if apply_mask:
            m["rmask"] = mask_cols(rm, c)
        in_maps1.append(m)
    res1 = _run(nc1, in_maps1, "l1")

    # host reduce in fp64 + hi/lo split
    kvh_b, kvl_b, ksh_b, ksl_b = [], [], [], []
    for b in range(B):
        kv = (res1[2 * b]["kv_part"].astype(np.float64)
              + res1[2 * b + 1]["kv_part"].astype(np.float64))
        ksum = (res1[2 * b]["ksum_acc"].astype(np.float64).sum(axis=0)
                + res1[2 * b + 1]["ksum_acc"].astype(np.float64).sum(axis=0))
        kvh, kvl = _split_hi_lo(kv)
        ksh, ksl = _split_hi_lo(ksum)
        kvh_b.append(kvh)
        kvl_b.append(kvl)
        # column layout [p, cc]: c = cc*128 + p
        ksh_b.append(np.ascontiguousarray(ksh.reshape(2, 128).T))
        ksl_b.append(np.ascontiguousarray(ksl.reshape(2, 128).T))

    # ---- L2: chosen pass ----
    nc2 = _build_l2()
    in_maps2 = []
    for c in range(N_CORES):
        b = c // 2
        in_maps2.append({
            "ct": shard_r(ch, c), "wq": wqr,
            "kvh": kvh_b[b], "kvl": kvl_b[b],
            "ksh": ksh_b[b], "ksl": ksl_b[b],
        })
    res2 = _run(nc2, in_maps2, "l2")
    return _host_tail(res2, cm, rm)


def _host_tail(res, cm, rm):
    """fp64 tail: rows -> masked mean -> tau -> clip."""
    taus = np.zeros(B, np.float64)
    for b in range(B):
        num = np.concatenate([res[2 * b + h]["num_out"].ravel() for h in (0, 1)])
        den = np.concatenate([res[2 * b + h]["den_out"].ravel() for h in (0, 1)])
        num = np.maximum(num.astype(np.float64), EPS)
        den = np.maximum((den.astype(np.float64) + EPS) ** 2, EPS)
        row = num / den
        cmb = cm[b].astype(np.float64)
        q_counts = max(cmb.sum(), 1.0)
        m = (row * cmb).sum() / q_counts
        n = max(rm[b].astype(np.float64).sum(), 1.0)
        tau = (n * m - 1.0) / max(n - 1.0, 1e-6)
        taus[b] = min(max(tau, 0.0), 1.0)
    return taus.astype(np.float32)


# revision 12
# speedup vs baseline: 1.1991x; 1.1991x over previous
"""TRN2 Bass kernel for nn_AttentionSimilarityLinear (B=4, S=8192, D=1024, C=256).

Sharding: 8 cores = (batch b = core//2, seq half h = core%2), 4096 rows each.
Two SPMD launches:
  L1 (rejected pass): K = elu(R @ W_K)+1 masked; outputs per-core partial
     KV = K^T K (fp32r matmuls, fp32 PSUM) and K_sum partition-partials.
  host: reduce KV/K_sum over core pairs in fp64; split into fp32r hi/lo pairs.
  L2 (chosen pass): Q^T = elu(C @ W_Q)+1; T = (KVh+KVl) @ Q^T;
     num[s] = sum_d (T .* Q^T); den[s] = Q^T . (Ksh+Ksl); outputs num/den rows.
  host tail in fp64: row = max(num,eps)/max((den+eps)^2,eps); masked mean; tau; clip.

All matmuls use fp32r (operands RNE-rounded to 11 mantissa bits, full-speed PE).
The hidden states and weights are pre-rounded on host (exactly matching the HW
rounding, verified bit-exact) and shipped as fp32r DRAM tensors, so no on-chip
rounding passes are needed. The ill-conditioned (n*m-1) cancellation runs on
host in fp64; the hi/lo split of KV/K_sum keeps the row-stat matmuls at ~2^-22
effective precision, holding end-to-end tau error at ~0.02% (simulated).
"""
import os
import numpy as np

import concourse.bacc as bacc
import concourse.tile as tile
from concourse import mybir
from concourse.bass_utils import run_bass_kernel_spmd

B, S, D, C = 4, 8192, 1024, 256
SH = S // 2              # rows per core
N_CORES = 8
EPS = 1e-8

F32 = mybir.dt.float32
F32R = mybir.dt.float32r
Relu = mybir.ActivationFunctionType.Relu
Exp = mybir.ActivationFunctionType.Exp

# group geometry
GW = 512                 # rows per DMA group
NG = SH // GW            # 8 groups
NT = GW // 128           # 4 tiles of 128 rows per group
DC = D // 128            # 8 contraction chunks

TRACE = bool(os.environ.get("KERNEL_TRACE"))
LAST_EXEC_NS = {}
_NC_CACHE = {}


def _rne11(x):
    """Round fp32 array to 11 explicit mantissa bits, RNE (matches HW fp32r)."""
    x = np.ascontiguousarray(x, np.float32)
    u = x.view(np.uint32)
    shift = np.uint32(12)  # 23 - 11
    low = u & np.uint32((1 << 12) - 1)
    half = np.uint32(1 << 11)
    base = u >> shift
    rnd = base + (((low > half) | ((low == half) & ((base & 1) == 1)))).astype(np.uint32)
    return (rnd << shift).view(np.float32).copy()


def _shard_blocked(x, core):
    """Shard rows for `core`, transpose to [D, SH], then block so each
    512-row group is one fully-contiguous 2MB DMA: [NG, 128, DC, GW] with
    element (g, p, dc, s) = X^T[dc*128+p, g*GW+s]."""
    b, h = divmod(core, 2)
    xt = _rne11(np.ascontiguousarray(x[b, h * SH:(h + 1) * SH, :].T))
    return np.ascontiguousarray(xt.reshape(DC, 128, NG, GW).transpose(2, 1, 0, 3))


def _split_hi_lo(x64):
    """fp64 array -> (hi, lo) fp32 arrays, both 11-bit-mantissa representable."""
    hi = _rne11(x64.astype(np.float32))
    lo = _rne11((x64 - hi.astype(np.float64)).astype(np.float32))
    return hi, lo


def _build_l1(apply_mask, reps=1):
    nc = bacc.Bacc()
    rt_d = nc.declare_dram_parameter("rt", [D, SH], F32R, isOutput=False)
    wk_d = nc.declare_dram_parameter("wk", [D, C], F32R, isOutput=False)
    if apply_mask:
        rmask_d = nc.declare_dram_parameter("rmask", [128, SH // 128], F32, isOutput=False)
    kv_d = nc.declare_dram_parameter("kv_part", [C, C], F32, isOutput=True)
    ksum_d = nc.declare_dram_parameter("ksum_acc", [128, C], F32, isOutput=True)

    with tile.TileContext(nc) as tc:
        with (
            tc.tile_pool(name="const", bufs=1) as const,
            tc.tile_pool(name="xin", bufs=3) as xin,
            tc.tile_pool(name="kt", bufs=3) as kt,
            tc.tile_pool(name="ps_k", bufs=4, space="PSUM") as ps_k_pool,
            tc.tile_pool(name="ps_kv", bufs=1, space="PSUM") as ps_kv_pool,
        ):
            wkr = const.tile([128, DC, C], F32R)
            nc.sync.dma_start(out=wkr, in_=wk_d[:].rearrange("(a p) c -> p a c", p=128))
            if apply_mask:
                rmask_sb = const.tile([128, SH // 128], F32)
                nc.sync.dma_start(out=rmask_sb, in_=rmask_d[:])

            ksum_acc = const.tile([128, C], F32)
            nc.vector.memset(ksum_acc, 0.0)

            kv_ps = [
                ps_kv_pool.tile([128, C], F32, tag=f"kv{cc}", name=f"kv_ps{cc}")
                for cc in range(2)
            ]

            for rep in range(reps):
              for g in range(NG):
                xtr = xin.tile([128, DC, GW], F32R, tag="xtr")
                nc.sync.dma_start(
                    out=xtr,
                    in_=rt_d[:, g * GW:(g + 1) * GW].rearrange("(a p) s -> p a s", p=128),
                )

                for t in range(NT):
                    ps_k = ps_k_pool.tile([128, C], F32)
                    for dc in range(DC):
                        nc.tensor.matmul(
                            ps_k,
                            lhsT=xtr[:, dc, t * 128:(t + 1) * 128],
                            rhs=wkr[:, dc, :],
                            start=(dc == 0),
                            stop=(dc == DC - 1),
                        )
                    idx = g * NT + t
                    gidx = rep * (SH // 128) + idx
                    # elu(x)+1 = relu(x) + exp(min(x, 0))
                    relu_t = kt.tile([128, C], F32, tag="relu")
                    nc.scalar.activation(out=relu_t, in_=ps_k, func=Relu)
                    mn_t = kt.tile([128, C], F32, tag="mn")
                    nc.vector.tensor_scalar_min(out=mn_t, in0=ps_k, scalar1=0.0)
                    ex_t = kt.tile([128, C], F32, tag="ex")
                    nc.scalar.activation(out=ex_t, in_=mn_t, func=Exp)
                    if apply_mask:
                        ke_t = kt.tile([128, C], F32, tag="ke")
                        nc.vector.tensor_add(out=ke_t, in0=relu_t, in1=ex_t)
                        kr = kt.tile([128, C], F32R, tag="kr")
                        nc.vector.tensor_scalar_mul(
                            out=kr, in0=ke_t, scalar1=rmask_sb[:, idx:idx + 1]
                        )
                    else:
                        kr = kt.tile([128, C], F32R, tag="kr")
                        nc.vector.tensor_add(out=kr, in0=relu_t, in1=ex_t)
                    nc.vector.tensor_add(out=ksum_acc, in0=ksum_acc, in1=kr)
                    first = gidx == 0
                    last = gidx == reps * (SH // 128) - 1
                    for cc in range(2):
                        nc.tensor.matmul(
                            kv_ps[cc],
                            lhsT=kr[:, cc * 128:(cc + 1) * 128],
                            rhs=kr[:, :],
                            start=first,
                            stop=last,
                        )

            for cc in range(2):
                kv_sb = kt.tile([128, C], F32, tag=f"kvout{cc}")
                nc.vector.tensor_copy(out=kv_sb, in_=kv_ps[cc])
                nc.sync.dma_start(out=kv_d[cc * 128:(cc + 1) * 128, :], in_=kv_sb)
            nc.sync.dma_start(out=ksum_d[:], in_=ksum_acc)

    nc.finalize()
    return nc


def _build_l2(reps=1):
    nc = bacc.Bacc()
    ct_d = nc.declare_dram_parameter("ct", [D, SH], F32R, isOutput=False)
    wq_d = nc.declare_dram_parameter("wq", [D, C], F32R, isOutput=False)
    kvh_d = nc.declare_dram_parameter("kvh", [C, C], F32R, isOutput=False)
    kvl_d = nc.declare_dram_parameter("kvl", [C, C], F32R, isOutput=False)
    ksh_d = nc.declare_dram_parameter("ksh", [128, 2], F32R, isOutput=False)
    ksl_d = nc.declare_dram_parameter("ksl", [128, 2], F32R, isOutput=False)
    num_d = nc.declare_dram_parameter("num_out", [1, SH], F32, isOutput=True)
    den_d = nc.declare_dram_parameter("den_out", [1, SH], F32, isOutput=True)

    with tile.TileContext(nc) as tc:
        with (
            tc.tile_pool(name="const", bufs=1) as const,
            tc.tile_pool(name="xin", bufs=3) as xin,
            tc.tile_pool(name="qt", bufs=2) as qt,
            tc.tile_pool(name="out", bufs=1) as outp,
            tc.tile_pool(name="ps_q", bufs=2, space="PSUM") as ps_q_pool,
            tc.tile_pool(name="ps_t", bufs=2, space="PSUM") as ps_t_pool,
            tc.tile_pool(name="ps_nd", bufs=2, space="PSUM") as ps_nd_pool,
        ):
            wqr = const.tile([128, DC, C], F32R)
            nc.sync.dma_start(out=wqr, in_=wq_d[:].rearrange("(a p) c -> p a c", p=128))

            # KV hi/lo: layout [p, cc, d] with c = cc*128 + p
            kv_r = []
            for name, d_in in (("kvh", kvh_d), ("kvl", kvl_d)):
                r = const.tile([128, 2, C], F32R, tag=f"{name}r", name=f"{name}_sb")
                nc.sync.dma_start(out=r, in_=d_in[:].rearrange("(a p) c -> p a c", p=128))
                kv_r.append(r)

            ks_r = []
            for name, d_in in (("ksh", ksh_d), ("ksl", ksl_d)):
                r = const.tile([128, 2], F32R, tag=f"{name}r", name=f"{name}_sb")
                nc.sync.dma_start(out=r, in_=d_in[:])
                ks_r.append(r)

            ones_sb = const.tile([128, 1], F32)
            nc.vector.memset(ones_sb, 1.0)
            ones_r = const.tile([128, 1], F32R)
            nc.vector.tensor_copy(out=ones_r, in_=ones_sb)

            num_sb = outp.tile([1, SH], F32)
            den_sb = outp.tile([1, SH], F32)

            for rep in range(reps):
              for g in range(NG):
                xtr = xin.tile([128, DC, GW], F32R, tag="xtr")
                nc.sync.dma_start(
                    out=xtr,
                    in_=ct_d[:, g * GW:(g + 1) * GW].rearrange("(a p) s -> p a s", p=128),
                )

                qtr = qt.tile([128, 2, GW], F32R, tag="qtr")
                for cc in range(2):
                    ps_q = ps_q_pool.tile([128, GW], F32)
                    for dc in range(DC):
                        nc.tensor.matmul(
                            ps_q,
                            lhsT=wqr[:, dc, cc * 128:(cc + 1) * 128],
                            rhs=xtr[:, dc, :],
                            start=(dc == 0),
                            stop=(dc == DC - 1),
                        )
                    relu_t = qt.tile([128, GW], F32, tag="relu")
                    nc.scalar.activation(out=relu_t, in_=ps_q, func=Relu)
                    mn_t = qt.tile([128, GW], F32, tag="mn")
                    nc.vector.tensor_scalar_min(out=mn_t, in0=ps_q, scalar1=0.0)
                    ex_t = qt.tile([128, GW], F32, tag="ex")
                    nc.scalar.activation(out=ex_t, in_=mn_t, func=Exp)
                    nc.vector.tensor_add(out=qtr[:, cc, :], in0=relu_t, in1=ex_t)

                ps_num = ps_nd_pool.tile([1, GW], F32, tag="num")
                ps_den = ps_nd_pool.tile([1, GW], F32, tag="den")
                for dd in range(2):
                    ps_t = ps_t_pool.tile([128, GW], F32)
                    mm = 0
                    for kv_t in kv_r:
                        for cc in range(2):
                            nc.tensor.matmul(
                                ps_t,
                                lhsT=kv_t[:, cc, dd * 128:(dd + 1) * 128],
                                rhs=qtr[:, cc, :],
                                start=(mm == 0),
                                stop=(mm == 3),
                            )
                            mm += 1
                    p2 = qt.tile([128, GW], F32R, tag="p2")
                    nc.vector.tensor_mul(out=p2, in0=ps_t, in1=qtr[:, dd, :])
                    nc.tensor.matmul(
                        ps_num, lhsT=ones_r, rhs=p2,
                        start=(dd == 0), stop=(dd == 1),
                    )
                mm = 0
                for ks_t in ks_r:
                    for cc in range(2):
                        nc.tensor.matmul(
                            ps_den,
                            lhsT=ks_t[:, cc:cc + 1],
                            rhs=qtr[:, cc, :],
                            start=(mm == 0),
                            stop=(mm == 3),
                        )
                        mm += 1
                nc.vector.tensor_copy(out=num_sb[:, g * GW:(g + 1) * GW], in_=ps_num)
                nc.vector.tensor_copy(out=den_sb[:, g * GW:(g + 1) * GW], in_=ps_den)

            nc.sync.dma_start(out=num_d[:], in_=num_sb)
            nc.sync.dma_start(out=den_d[:], in_=den_sb)

    nc.finalize()
    return nc


def _build_fused(apply_mask, reps=1):
    """Single launch: rejected pass -> pairwise AllReduce of (KV, Ksum) ->
    on-device hi/lo split -> chosen pass. Chosen-side DMA/projections overlap
    the rejected pass and the collective."""
    nc = bacc.Bacc()
    rt_d = nc.declare_dram_parameter("rt", [NG, 128, DC, GW], F32R, isOutput=False)
    ct_d = nc.declare_dram_parameter("ct", [NG, 128, DC, GW], F32R, isOutput=False)
    wk_d = nc.declare_dram_parameter("wk", [D, C], F32R, isOutput=False)
    wq_d = nc.declare_dram_parameter("wq", [D, C], F32R, isOutput=False)
    if apply_mask:
        rmask_d = nc.declare_dram_parameter("rmask", [128, SH // 128], F32, isOutput=False)
    num_d = nc.declare_dram_parameter("num_out", [1, SH], F32, isOutput=True)
    den_d = nc.declare_dram_parameter("den_out", [1, SH], F32, isOutput=True)
    cc_in = nc.dram_tensor("cc_in", [C + 1, C], F32)
    cc_out = nc.dram_tensor("cc_out", [C + 1, C], F32)
    groups = [[2 * b, 2 * b + 1] for b in range(B)]

    with tile.TileContext(nc) as tc:
        with (
            tc.tile_pool(name="const", bufs=1) as const,
            tc.tile_pool(name="xin", bufs=3) as xin,
            tc.tile_pool(name="kt", bufs=3) as kt,
            tc.tile_pool(name="qt", bufs=2) as qt,
            tc.tile_pool(name="out", bufs=1) as outp,
            tc.tile_pool(name="ps_mm", bufs=2, space="PSUM") as ps_mm_pool,
            tc.tile_pool(name="ps_kv", bufs=1, space="PSUM") as ps_kv_pool,
            tc.tile_pool(name="ps_t", bufs=2, space="PSUM") as ps_t_pool,
            tc.tile_pool(name="ps_nd", bufs=1, space="PSUM") as ps_nd_pool,
        ):
            wkr = const.tile([128, DC, C], F32R)
            nc.sync.dma_start(out=wkr, in_=wk_d[:].rearrange("(a p) c -> p a c", p=128))
            wqr = const.tile([128, DC, C], F32R)
            nc.sync.dma_start(out=wqr, in_=wq_d[:].rearrange("(a p) c -> p a c", p=128))
            if apply_mask:
                rmask_sb = const.tile([128, SH // 128], F32)
                nc.sync.dma_start(out=rmask_sb, in_=rmask_d[:])

            ones_sb = const.tile([128, 1], F32)
            nc.vector.memset(ones_sb, 1.0)
            ones_r = const.tile([128, 1], F32R)
            nc.vector.tensor_copy(out=ones_r, in_=ones_sb)

            ksum_acc = const.tile([128, C], F32)
            nc.vector.memset(ksum_acc, 0.0)
            kv_ps = [
                ps_kv_pool.tile([128, C], F32, tag=f"kv{cc}", name=f"kv_ps{cc}")
                for cc in range(2)
            ]

            # ---- rejected pass ----
            for rep in range(reps):
              for g in range(NG):
                xtr = xin.tile([128, DC, GW], F32R, tag="xtr")
                nc.sync.dma_start(out=xtr, in_=rt_d[:][g])
                for t in range(NT):
                    ps_k = ps_mm_pool.tile([128, GW], F32, tag="mm", name="ps_k")
                    ps_k = ps_k[:, :C]
                    for dc in range(DC):
                        nc.tensor.matmul(
                            ps_k,
                            lhsT=xtr[:, dc, t * 128:(t + 1) * 128],
                            rhs=wkr[:, dc, :],
                            start=(dc == 0),
                            stop=(dc == DC - 1),
                        )
                    idx = g * NT + t
                    gidx = rep * (SH // 128) + idx
                    relu_t = kt.tile([128, C], F32, tag="relu")
                    nc.scalar.activation(out=relu_t, in_=ps_k, func=Relu)
                    mn_t = kt.tile([128, C], F32, tag="mn")
                    nc.vector.tensor_scalar_min(out=mn_t, in0=ps_k, scalar1=0.0)
                    ex_t = kt.tile([128, C], F32, tag="ex")
                    nc.scalar.activation(out=ex_t, in_=mn_t, func=Exp)
                    if apply_mask:
                        ke_t = kt.tile([128, C], F32, tag="ke")
                        nc.vector.tensor_add(out=ke_t, in0=relu_t, in1=ex_t)
                        kr = kt.tile([128, C], F32R, tag="kr")
                        nc.vector.tensor_scalar_mul(
                            out=kr, in0=ke_t, scalar1=rmask_sb[:, idx:idx + 1]
                        )
                    else:
                        kr = kt.tile([128, C], F32R, tag="kr")
                        nc.vector.tensor_add(out=kr, in0=relu_t, in1=ex_t)
                    nc.vector.tensor_add(out=ksum_acc, in0=ksum_acc, in1=kr)
                    first = gidx == 0
                    last = gidx == reps * (SH // 128) - 1
                    for cc in range(2):
                        nc.tensor.matmul(
                            kv_ps[cc],
                            lhsT=kr[:, cc * 128:(cc + 1) * 128],
                            rhs=kr[:, :],
                            start=first,
                            stop=last,
                        )

            # ---- reduce K stats across the core pair ----
            for cc in range(2):
                kv_sb = kt.tile([128, C], F32, tag=f"kvout{cc}")
                nc.vector.tensor_copy(out=kv_sb, in_=kv_ps[cc])
                nc.sync.dma_start(out=cc_in[cc * 128:(cc + 1) * 128, :], in_=kv_sb)
            # exact fp32 partition-reduce of ksum_acc
            ps_ks = ps_nd_pool.tile([1, C], F32, tag="num", name="ps_ks")
            nc.tensor.matmul(ps_ks, lhsT=ones_sb, rhs=ksum_acc, start=True, stop=True)
            ks_row = kt.tile([1, C], F32, tag="ksrow")
            nc.vector.tensor_copy(out=ks_row, in_=ps_ks)
            nc.sync.dma_start(out=cc_in[C:C + 1, :], in_=ks_row)

            nc.gpsimd.collective_compute(
                "AllReduce", mybir.AluOpType.add,
                replica_groups=groups,
                ins=[cc_in[:]], outs=[cc_out[:]],
            )

            kvsum_sb = const.tile([128, 2, C], F32)
            nc.sync.dma_start(
                out=kvsum_sb, in_=cc_out[0:C, :].rearrange("(a p) c -> p a c", p=128)
            )
            kvh_r = const.tile([128, 2, C], F32R)
            nc.vector.tensor_copy(out=kvh_r, in_=kvsum_sb)
            kvl_r = const.tile([128, 2, C], F32R)
            nc.vector.tensor_sub(out=kvl_r, in0=kvsum_sb, in1=kvh_r)
            kv_r = [kvh_r, kvl_r]

            ksc_sb = const.tile([128, 2], F32)
            nc.sync.dma_start(
                out=ksc_sb, in_=cc_out[C, :].rearrange("(a p) -> p a", p=128)
            )
            ksh_r = const.tile([128, 2], F32R)
            nc.vector.tensor_copy(out=ksh_r, in_=ksc_sb)
            ksl_r = const.tile([128, 2], F32R)
            nc.vector.tensor_sub(out=ksl_r, in0=ksc_sb, in1=ksh_r)
            ks_r = [ksh_r, ksl_r]

            # ---- chosen pass ----
            num_sb = outp.tile([1, SH], F32)
            den_sb = outp.tile([1, SH], F32)
            for rep in range(reps):
              for g in range(NG):
                xtc = xin.tile([128, DC, GW], F32R, tag="xtc", name="xtc")
                nc.sync.dma_start(out=xtc, in_=ct_d[:][g])
                qtr = qt.tile([128, 2, GW], F32R, tag="qtr")
                for cc in range(2):
                    ps_q = ps_mm_pool.tile([128, GW], F32, tag="mm", name="ps_q")
                    for dc in range(DC):
                        nc.tensor.matmul(
                            ps_q,
                            lhsT=wqr[:, dc, cc * 128:(cc + 1) * 128],
                            rhs=xtc[:, dc, :],
                            start=(dc == 0),
                            stop=(dc == DC - 1),
                        )
                    relu_t = qt.tile([128, GW], F32, tag="qrelu")
                    nc.scalar.activation(out=relu_t, in_=ps_q, func=Relu)
                    mn_t = qt.tile([128, GW], F32, tag="qmn")
                    nc.vector.tensor_scalar_min(out=mn_t, in0=ps_q, scalar1=0.0)
                    ex_t = qt.tile([128, GW], F32, tag="qex")
                    nc.scalar.activation(out=ex_t, in_=mn_t, func=Exp)
                    nc.vector.tensor_add(out=qtr[:, cc, :], in0=relu_t, in1=ex_t)

                ps_num = ps_nd_pool.tile([1, GW], F32, tag="num", name="ps_num")
                ps_den = ps_nd_pool.tile([1, GW], F32, tag="den", name="ps_den")
                for dd in range(2):
                    ps_t = ps_t_pool.tile([128, GW], F32)
                    mm = 0
                    for kv_t in kv_r:
                        for cc in range(2):
                            nc.tensor.matmul(
                                ps_t,
                                lhsT=kv_t[:, cc, dd * 128:(dd + 1) * 128],
                                rhs=qtr[:, cc, :],
                                start=(mm == 0),
                                stop=(mm == 3),
                            )
                            mm += 1
                    p2 = qt.tile([128, GW], F32R, tag="p2")
                    nc.vector.tensor_mul(out=p2, in0=ps_t, in1=qtr[:, dd, :])
                    nc.tensor.matmul(
                        ps_num, lhsT=ones_r, rhs=p2,
                        start=(dd == 0), stop=(dd == 1),
                    )
                mm = 0
                for ks_t in ks_r:
                    for cc in range(2):
                        nc.tensor.matmul(
                            ps_den,
                            lhsT=ks_t[:, cc:cc + 1],
                            rhs=qtr[:, cc, :],
                            start=(mm == 0),
                            stop=(mm == 3),
                        )
                        mm += 1
                nc.vector.tensor_copy(out=num_sb[:, g * GW:(g + 1) * GW], in_=ps_num)
                nc.vector.tensor_copy(out=den_sb[:, g * GW:(g + 1) * GW], in_=ps_den)

            nc.sync.dma_start(out=num_d[:], in_=num_sb)
            nc.sync.dma_start(out=den_d[:], in_=den_sb)

    nc.finalize()
    return nc


def _run(nc, in_maps, label):
    kwargs = {}
    if TRACE:
        tmpdir = f"/tmp/kernel_trace_{label}"
        os.makedirs(tmpdir, exist_ok=True)
        kwargs = dict(trace=True, tmpdir=tmpdir)
    res = run_bass_kernel_spmd(nc, in_maps, core_ids=list(range(N_CORES)), **kwargs)
    if TRACE:
        LAST_EXEC_NS[label] = res.exec_time_ns
    return res.results


def kernel(chosen_hidden_states, rejected_hidden_states, chosen_mask,
           rejected_mask, W_Q, W_K):
    ch = np.ascontiguousarray(chosen_hidden_states, np.float32)
    rh = np.ascontiguousarray(rejected_hidden_states, np.float32)
    cm = np.ascontiguousarray(chosen_mask, np.float32)
    rm = np.ascontiguousarray(rejected_mask, np.float32)
    wqr = _rne11(np.ascontiguousarray(W_Q, np.float32))
    wkr = _rne11(np.ascontiguousarray(W_K, np.float32))

    def shard_r(x, core):
        b, h = divmod(core, 2)
        return _rne11(np.ascontiguousarray(x[b, h * SH:(h + 1) * SH, :].T))

    def mask_cols(m, core):
        b, h = divmod(core, 2)
        return np.ascontiguousarray(m[b, h * SH:(h + 1) * SH].reshape(SH // 128, 128).T)

    apply_mask = not np.all(rm == 1.0)

    # ---- preferred: fused single launch with pairwise AllReduce ----
    try:
        key = ("fused", apply_mask)
        nc = _NC_CACHE.get(key)
        if nc is None:
            nc = _build_fused(apply_mask)
            _NC_CACHE[key] = nc
        in_maps = []
        for c in range(N_CORES):
            m = {"rt": _shard_blocked(rh, c), "ct": _shard_blocked(ch, c),
                 "wk": wkr, "wq": wqr}
            if apply_mask:
                m["rmask"] = mask_cols(rm, c)
            in_maps.append(m)
        res = _run(nc, in_maps, "fused")
        return _host_tail(res, cm, rm)
    except Exception as e:  # pragma: no cover - fallback for safety
        import traceback
        traceback.print_exc()
        print(f"fused kernel failed ({e!r}); falling back to two-launch path")

    # ---- L1: rejected pass ----
    nc1 = _build_l1(apply_mask)
    in_maps1 = []
    for c in range(N_CORES):
        m = {"rt": shard_r(rh, c), "wk": wkr}
        if apply_mask:
            m["rmask"] = mask_cols(rm, c)
        in_maps1.append(m)
    res1 = _run(nc1, in_maps1, "l1")

    # host reduce in fp64 + hi/lo split
    kvh_b, kvl_b, ksh_b, ksl_b = [], [], [], []
    for b in range(B):
        kv = (res1[2 * b]["kv_part"].astype(np.float64)
              + res1[2 * b + 1]["kv_part"].astype(np.float64))
        ksum = (res1[2 * b]["ksum_acc"].astype(np.float64).sum(axis=0)
                + res1[2 * b + 1]["ksum_acc"].astype(np.float64).sum(axis=0))
        kvh, kvl = _split_hi_lo(kv)
        ksh, ksl = _split_hi_lo(ksum)
        kvh_b.append(kvh)
        kvl_b.append(kvl)
        # column layout [p, cc]: c = cc*128 + p
        ksh_b.append(np.ascontiguousarray(ksh.reshape(2, 128).T))
        ksl_b.append(np.ascontiguousarray(ksl.reshape(2, 128).T))

    # ---- L2: chosen pass ----
    nc2 = _build_l2()
    in_maps2 = []
    for c in range(N_CORES):
        b = c // 2
        in_maps2.append({
            "ct": shard_r(ch, c), "wq": wqr,
            "kvh": kvh_b[b], "kvl": kvl_b[b],
            "ksh": ksh_b[b], "ksl": ksl_b[b],
        })
    res2 = _run(nc2, in_maps2, "l2")
    return _host_tail(res2, cm, rm)


def _host_tail(res, cm, rm):
    """fp64 tail: rows -> masked mean -> tau -> clip."""
    taus = np.zeros(B, np.float64)
    for b in range(B):
        num = np.concatenate([res[2 * b + h]["num_out"].ravel() for h in (0, 1)])
        den = np.concatenate([res[2 * b + h]["den_out"].ravel() for h in (0, 1)])
        num = np.maximum(num.astype(np.float64), EPS)
        den = np.maximum((den.astype(np.float64) + EPS) ** 2, EPS)
        row = num / den
        cmb = cm[b].astype(np.float64)
        q_counts = max(cmb.sum(), 1.0)
        m = (row * cmb).sum() / q_counts
        n = max(rm[b].astype(np.float64).sum(), 1.0)
        tau = (n * m - 1.0) / max(n - 1.0, 1e-6)
        taus[b] = min(max(tau, 0.0), 1.0)
    return taus.astype(np.float32)
